# revision 5
# baseline (speedup 1.0000x reference)
"""Trainium2 Bass kernel for causal+padded multi-head attention.

Problem: B=2, N=2048, D=1024, H=16 heads (DK=64), fp32 I/O.
  out = softmax(mask(x Wq^T (x Wk^T)^T) / sqrt(DK)) (x Wv^T) Wout^T + b_out

Sharding (8 cores): core c handles batch b=c//4 and heads [4*(c%4), 4*(c%4)+4).
Each core computes a partial output [N, D] (its 4 heads' contribution through
the output projection, bf16); the host sums the 4 partials per batch in fp32
and adds b_out.

Schedule (v2, from the 156us baseline's trace):
  - PE p-state: TRN2 throttles the PE clock to 1.2/0.65 GHz after idle and
    only reaches 2.4 GHz after ~3us of continuous execution.  A chain of
    warm-up matmuls on a memset tile runs during the DMA head so real work
    starts at full clock, and the schedule keeps the PE dense to stay there.
  - Input DMAs are split per-128-row chunk (wq/wk/wv per e, xt per (e, qtile))
    and interleaved across BOTH hardware queues (SP + ACT) so the first
    projection matmul can start ~1us after the queues spin up instead of
    after the full weight load.
  - Attention units (pair, qtile) riffle their OWN PV matmuls into the S^T
    stream at a small lag (the PSUM ctx banks of the previous unit are
    released by its normalize, which is emitted just before this unit).
    Unit order ends on a 4-chunk qt=0 unit so the tail chain after the last
    exp is short.
  - Normalize is split into phases: the DVE transpose/reciprocal dance at the
    unit boundary, the GpSimd partition-broadcast right after, and the DVE
    multiplies flushed 2 chunks into the next unit's stream -- so the 1us
    broadcast latency never stalls the in-order DVE queue in front of the
    round evacuations that pace the PE.
  - The LAST unit's normalize runs in 256-column halves with the final
    out-projection rounds interleaved, and the final out DMAs are split per
    512-column half and alternated across both queues.

Known dead ends (measured): fp8 anywhere gives 4-8e-2 rel err (gate 1e-2);
reciprocal_approx_fast (custom DVE uop) returns garbage/crashes the exec unit
under this runtime; exp(-ln(den)) on ScalarE forces an ACT table swap per
call (~38us total); SBUF->SBUF DMA cannot cross partitions, and DRAM-bounce
transposes race (DMA queue issue is async, Tile does not serialize the DRAM
RAW).
"""

import math
import os

import numpy as np

B, N, D, H = 2, 2048, 1024, 16
DK = D // H  # 64
NCORES = 8
HEADS_PER_CORE = 4
QTILE = 512
KBLK = 128
NEG = -30000.0
NEGB = -3750.0  # pad bias applied after the 0.125 scale inside exp
SCALE = 1.0 / math.sqrt(float(DK))  # 0.125
RIFFLE_LAG = 3  # PV chunk j rides the same unit's S^T stream after chunk j+LAG
NWARM = 10  # PE clock warm-up matmuls during the DMA head

# Set by run() when tracing is enabled (test.py reads this).
LAST_RESULTS = None


def _build_program(kb_max: int, jpad_min: int):
    import concourse.tile as tile
    from concourse import bacc, mybir

    F32 = mybir.dt.float32
    BF16 = mybir.dt.bfloat16
    EXP = mybir.ActivationFunctionType.Exp
    ADD = mybir.AluOpType.add

    nc = bacc.Bacc(None)

    xt_d = nc.dram_tensor("xt", [D, N], BF16, kind="ExternalInput")
    wq_d = nc.dram_tensor("wq", [D, 256], BF16, kind="ExternalInput")
    wk_d = nc.dram_tensor("wk", [D, 256], BF16, kind="ExternalInput")
    wv_d = nc.dram_tensor("wv", [D, 256], BF16, kind="ExternalInput")
    wout_d = nc.dram_tensor("wout", [256, D], BF16, kind="ExternalInput")
    padb_d = nc.dram_tensor("padbias", [128, 16], F32, kind="ExternalInput")
    trineg_d = nc.dram_tensor("trineg", [128, 896], BF16, kind="ExternalInput")
    out_d = nc.dram_tensor("out", [N, D], BF16, kind="ExternalOutput")

    NB = N // KBLK  # 16 key/row blocks
    NQT = N // QTILE  # 4 q tiles

    with tile.TileContext(nc) as tc:
        with (
            tc.tile_pool(name="w", bufs=1) as w_pool,
            tc.tile_pool(name="big", bufs=1) as big_pool,
            tc.tile_pool(name="work", bufs=3) as work_pool,
            tc.tile_pool(name="osb", bufs=3) as osb_pool,
            tc.tile_pool(name="xt", bufs=1) as xt_pool,
            tc.tile_pool(name="pt", bufs=10) as pt_pool,
            tc.tile_pool(name="ps_st", bufs=2, space="PSUM") as ps_st,
            tc.tile_pool(name="ps_b", bufs=2, space="PSUM") as ps_b,
            tc.tile_pool(name="ps_ctx", bufs=1, space="PSUM") as ps_ctx,
        ):
            # ---- SBUF tiles ------------------------------------------------
            wq8 = [w_pool.tile([128, 256], BF16, tag=f"wq{e}", name=f"wq{e}") for e in range(8)]
            wk8 = [w_pool.tile([128, 256], BF16, tag=f"wk{e}", name=f"wk{e}") for e in range(8)]
            wv8 = [w_pool.tile([128, 256], BF16, tag=f"wv{e}", name=f"wv{e}") for e in range(8)]
            wo2 = [w_pool.tile([128, D], BF16, tag=f"wo{c}", name=f"wo{c}") for c in range(2)]
            padb_t = w_pool.tile([128, 16], F32, tag="padb", name="padb")
            trineg_t = w_pool.tile([128, 896], BF16, tag="trineg", name="trineg")
            xt = [[xt_pool.tile([128, 512], BF16, tag=f"xt{e}_{c}", name=f"xt{e}_{c}")
                   for c in range(NQT)] for e in range(8)]

            # PE clock warm-up: a dense chain of matmuls on a memset tile
            # runs during the DMA head so the p-state ramp (0.65 -> 1.2 ->
            # 2.4 GHz over ~3us of continuous execution) finishes before the
            # first real matmul.
            warmw = w_pool.tile([128, 512], BF16, tag="warmw", name="warmw")
            nc.vector.memset(warmw[:], 0.03)
            for _ in range(NWARM):
                wps = ps_b.tile([128, 512], F32, tag="b", name="b")
                nc.tensor.matmul(wps[:], warmw[:, 0:128], warmw[:],
                                 start=True, stop=True)

            # ---- input DMAs (critical-path-first, both queues) -------------
            def q_of(i):
                return nc.sync if i % 2 == 0 else nc.scalar

            for e in range(8):
                q_of(e).dma_start(wq8[e][:], wq_d[e * 128:(e + 1) * 128, :])
                q_of(e).dma_start(xt[e][0][:], xt_d[e * 128:(e + 1) * 128, 0:512])
            for e in range(8):
                q_of(e).dma_start(wk8[e][:], wk_d[e * 128:(e + 1) * 128, :])
            nc.scalar.dma_start(trineg_t[:], trineg_d[:])
            for e in range(8):
                q_of(e).dma_start(wv8[e][:], wv_d[e * 128:(e + 1) * 128, :])
            for c in range(1, NQT):
                for e in range(8):
                    q_of(e + c).dma_start(
                        xt[e][c][:], xt_d[e * 128:(e + 1) * 128, c * 512:(c + 1) * 512]
                    )
                if c == 1:
                    nc.sync.dma_start(padb_t[:], padb_d[:])
            for c in range(2):
                q_of(c).dma_start(wo2[c][:], wout_d[c * 128:(c + 1) * 128, :])

            # warm the ACT exp table during the DMA head (AFTER the scalar
            # queue's dma_start instructions -- the ~2.7us table load would
            # otherwise block their issue)
            warm = work_pool.tile([1, 8], F32, tag="warm", name="warm")
            nc.vector.memset(warm[:], 1.0)
            nc.scalar.activation(warm[:], warm[:], EXP)

            # V' tile: [keys 128, key-block, head 4, 65]; col 64 <- ones so
            # P@V' also yields the softmax denominator on ctx row 64.
            v4 = big_pool.tile([128, kb_max, 4, 65], BF16, tag="v4", name="v4")
            nc.gpsimd.memset(v4[:, :, :, 64:65], 1.0)

            qt_pair = [big_pool.tile([128, N], BF16, tag=f"qt{p}", name=f"qt{p}") for p in range(2)]
            kt_pair = [big_pool.tile([128, N], BF16, tag=f"kt{p}", name=f"kt{p}") for p in range(2)]
            ctx_pair = [big_pool.tile([128, N], BF16, tag=f"ctx{p}", name=f"ctx{p}") for p in range(2)]

            # ---- PE filler rounds (projections / V / out-projection) -------
            pe_ns = [0.0]  # emitted PE work (ns)
            act_ns = [0.0]  # emitted ACT work (ns)

            def qk_round(w8, pair, nq, dst):
                ps = ps_b.tile([128, 512], F32, tag="b", name="b")
                for e in range(8):
                    nc.tensor.matmul(
                        ps[:],
                        w8[e][:, pair * 128:(pair + 1) * 128],
                        xt[e][nq][:],
                        start=(e == 0),
                        stop=(e == 7),
                    )
                nc.vector.tensor_copy(dst[pair][:, nq * 512:(nq + 1) * 512], ps[:])
                pe_ns[0] += 8 * 512 / 2.4

            def v_round(nb):
                ps = ps_b.tile([128, 512], F32, tag="b", name="b")[:, 0:256]
                c, coff = divmod(nb, 4)
                for e in range(8):
                    nc.tensor.matmul(
                        ps[:],
                        xt[e][c][:, coff * 128:(coff + 1) * 128],
                        wv8[e][:],
                        start=(e == 0),
                        stop=(e == 7),
                    )
                nc.vector.tensor_copy(
                    v4[:, nb, :, 0:64], ps[:].rearrange("p (h d) -> p h d", h=4)
                )
                pe_ns[0] += 8 * 256 / 2.4

            osb_tiles = {}

            def o_round(nb, fc):
                if fc == 0:
                    osb_tiles[nb] = osb_pool.tile([128, D], BF16, tag="osb", name="osb")
                osb = osb_tiles[nb]
                ps = ps_b.tile([128, 512], F32, tag="b", name="b")
                for pr2 in range(2):
                    nc.tensor.matmul(
                        ps[:],
                        ctx_pair[pr2][:, nb * 128:(nb + 1) * 128],
                        wo2[pr2][:, fc * 512:(fc + 1) * 512],
                        start=(pr2 == 0),
                        stop=(pr2 == 1),
                    )
                nc.vector.tensor_copy(osb[:, fc * 512:(fc + 1) * 512], ps[:])
                # per-half DMA on alternating queues: smaller final drain
                q = nc.sync if (nb + fc) % 2 == 0 else nc.scalar
                q.dma_start(
                    out_d[nb * 128:(nb + 1) * 128, fc * 512:(fc + 1) * 512],
                    osb[:, fc * 512:(fc + 1) * 512],
                )
                if fc == 1:
                    del osb_tiles[nb]
                pe_ns[0] += 2 * 512 / 2.4

            rounds = {}
            for pair in range(2):
                for nq in range(NQT):
                    rounds[("q", pair, nq)] = (lambda p=pair, n=nq: qk_round(wq8, p, n, qt_pair))
                    rounds[("k", pair, nq)] = (lambda p=pair, n=nq: qk_round(wk8, p, n, kt_pair))
            for nb in range(kb_max):
                rounds[("v", nb)] = (lambda n=nb: v_round(n))
            for nb in range(NB):
                for fc in range(2):
                    rounds[("o", nb, fc)] = (lambda n=nb, f=fc: o_round(n, f))

            emitted = set()
            filler_q = []

            def emit_rid(rid):
                if rid in emitted:
                    return
                emitted.add(rid)
                rounds[rid]()

            def inject_fillers(headroom=4000.0):
                # keep ~4us of emitted-but-unexecuted PE work beyond the ACT
                # frontier so the PE (the critical engine) never drains
                while filler_q and pe_ns[0] < act_ns[0] + headroom:
                    emit_rid(filler_q.pop(0))

            # ---- normalize -------------------------------------------------
            # The DVE reciprocal is an iterative 8-cyc/element divide
            # streaming the FREE dim, and the denominator row is 512 elements
            # on ONE partition.  Use the DVE 32x32 StreamTranspose to fold
            # the row onto 32 partitions, take the reciprocal 16-wide, fold
            # back (bf16: ~0.4% rms on the normalize scale), partition-
            # broadcast on GpSimd, and multiply ctx (read straight from
            # PSUM) by the broadcast row.
            def norm_phase_a(pair, hh, qt, ctx_ps, c0, cw):
                """transpose/recip/transpose + gpsimd broadcast for columns
                [c0, c0+cw); returns the rbr tile for the multiply."""
                nblk = cw // 32
                tscat = work_pool.tile([32, 512], F32, tag="tscat", name="tscat")
                nc.vector.transpose(tscat[:, 0:cw], ctx_ps[64:96, c0:c0 + cw])
                rscat = work_pool.tile([32, 512], BF16, tag="rscat", name="rscat")
                with nc.allow_low_precision(
                    reason="bf16 softmax-denominator reciprocal: ~0.4% rms "
                    "on the normalize scale, inside the error budget"
                ):
                    nc.vector.reciprocal(
                        rscat[:, 0:cw].rearrange("p (b s) -> p b s", s=32)[:, :, 0],
                        tscat[:, 0:cw].rearrange("p (b s) -> p b s", s=32)[:, :, 0],
                    )
                rrow = work_pool.tile([32, 512], BF16, tag="rrow", name="rrow")
                nc.vector.transpose(rrow[:, 0:cw], rscat[:, 0:cw])
                act_ns[0] += 2 * (cw + 352) / 1.2
                rbr = work_pool.tile([64, 512], BF16, tag="rbr", name="rbr")
                # GpSimd runs ONLY partition_broadcast ops (+ the startup
                # memsets): op-type churn makes walrus swap the firmware
                # library (~7us per swap)
                nc.gpsimd.partition_broadcast(rbr[:, 0:cw], rrow[0:1, 0:cw])
                return rbr

            def norm_phase_b(pair, hh, qt, ctx_ps, rbr, c0, cw):
                hp = slice(64 * hh, 64 * hh + 64)
                nc.vector.tensor_mul(
                    ctx_pair[pair][hp, qt * 512 + c0:qt * 512 + c0 + cw],
                    ctx_ps[0:64, c0:c0 + cw],
                    rbr[:, 0:cw],
                )

            done_norms = {q: 0 for q in range(NQT)}

            def note_norm_done(nqt):
                done_norms[nqt] += 1
                if done_norms[nqt] == 2:
                    for nb in range(4 * nqt, 4 * nqt + 4):
                        filler_q.append(("o", nb, 0))
                        filler_q.append(("o", nb, 1))

            # ---- attention unit: S^T + exp stream with own PV riffled ------
            def emit_unit(pair, qt, nchunks, pending_muls):
                ctx2 = [
                    ps_ctx.tile([96, 512], F32, tag=f"ctx{hh}", name=f"ctx{hh}")
                    for hh in range(2)
                ]
                pvq = []

                def pv_chunk(j, ptt, off):
                    for hh in range(2):
                        nc.tensor.matmul(
                            ctx2[hh][0:65, off:],
                            v4[:, j, 2 * pair + hh, :],
                            ptt[:, hh, off:],
                            start=(j == 0),
                            stop=(j == nchunks - 1),
                            skip_group_check=True,
                        )
                    pe_ns[0] += 2 * (512 - off) / 2.4

                for j in range(nchunks):
                    if j == 2 and pending_muls:
                        # prev unit's normalize multiplies, flushed after the
                        # gpsimd broadcasts have had ~2 chunks to complete so
                        # the in-order DVE queue never stalls on them
                        for fn in pending_muls:
                            fn()
                        pending_muls.clear()
                    while pvq and pvq[0][0] <= j - RIFFLE_LAG:
                        pv_chunk(*pvq.pop(0))
                    inject_fillers()
                    d = j - 4 * qt
                    # exact-causal column trim (keep matmul N >= 128)
                    off = 128 * d if d >= 1 else 0
                    st_ps = ps_st.tile([128, 2, 512], F32, tag="blk", name="blk")
                    for hh in range(2):
                        hp = slice(64 * hh, 64 * hh + 64)
                        nc.tensor.matmul(
                            st_ps[:, hh, off:],
                            kt_pair[pair][hp, j * 128:(j + 1) * 128],
                            qt_pair[pair][hp, qt * 512 + off:(qt + 1) * 512],
                            start=True,
                            stop=True,
                        )
                    pe_ns[0] += (512 - off) / 2.4
                    if d >= 0:
                        # causal add -30000; with off = 128*d the masked
                        # triangle lies entirely in cols [off, off+128);
                        # one op covers both heads via a stride-0 broadcast
                        u0 = 384 - 128 * d + off
                        w = min(128, 512 - off)
                        nc.vector.tensor_tensor(
                            st_ps[:, :, off:off + w],
                            st_ps[:, :, off:off + w],
                            trineg_t[:, u0:u0 + w].unsqueeze(1).broadcast_to(
                                (128, 2, w)
                            ),
                            ADD,
                        )
                    pt_t = pt_pool.tile([128, 2, 512], BF16, tag="pt", name="pt")
                    kw = {}
                    if j >= jpad_min:  # per-key pad bias (same for both heads)
                        kw["bias"] = padb_t[:, j:j + 1]
                    nc.scalar.activation(
                        pt_t[:, :, off:], st_ps[:, :, off:], EXP, scale=SCALE, **kw
                    )
                    act_ns[0] += (2 * (512 - off) + 352) / 1.2
                    pvq.append((j, pt_t, off))
                if pending_muls:
                    for fn in pending_muls:
                        fn()
                    pending_muls.clear()
                while pvq:
                    pv_chunk(*pvq.pop(0))
                return ctx2

            # Unit order: start with the cheapest unit (least input DMA),
            # end with a 4-chunk unit so the post-last-exp tail (PV drain +
            # normalize + o-rounds + out DMA) is short.
            units = [(0, 0), (0, 1), (1, 1), (0, 2), (1, 2), (0, 3), (1, 3), (1, 0)]
            units = [(p, qt, min(4 * qt + 4, kb_max)) for (p, qt) in units]

            # projection/V rounds become filler, ordered by the deadline of
            # the unit that first needs them (emit_rid dedups, so rounds the
            # unit loop hard-emits are simply skipped here)
            seen_rounds = set()
            for (p, qt, nch) in units:
                for rid in ([("q", p, qt)]
                            + [("k", p, nq) for nq in range(qt + 1)]
                            + [("v", nb) for nb in range(nch)]):
                    if rid not in seen_rounds:
                        seen_rounds.add(rid)
                        filler_q.append(rid)

            pending_muls = []
            prev = None  # (pair, qt, ctx2)
            for idx, (pair, qt, nchunks) in enumerate(units):
                # HARD-emit this unit's projection/V prereqs before any of
                # its S^T/PV instructions.  A read emitted before its writer
                # gets NO dependency from the Tile tracker (emission-ordered)
                # and would consume uninitialized SBUF; filler pacing alone
                # must never be trusted for correctness.
                emit_rid(("q", pair, qt))
                for nq in range(qt + 1):
                    emit_rid(("k", pair, nq))
                for nb in range(nchunks):
                    emit_rid(("v", nb))
                if prev is not None:
                    # previous unit's normalize: dance+broadcast now, the
                    # multiplies 2 chunks into this unit's stream.  This MUST
                    # precede emit_unit: ps_ctx has bufs=1, so this unit's PV
                    # start-write reuses the previous ctx banks and the Tile
                    # tracker (emission-ordered) only serializes them if the
                    # normalize reads are emitted first.
                    ppair, pqt, pctx2 = prev
                    for hh in range(2):
                        rbr = norm_phase_a(ppair, hh, pqt, pctx2[hh], 0, 512)
                        pending_muls.append(
                            (lambda p=ppair, h=hh, q=pqt, c=pctx2[hh], r=rbr:
                             norm_phase_b(p, h, q, c, r, 0, 512))
                        )
                    pending_muls.append(lambda q=pqt: note_norm_done(q))
                ctx2 = emit_unit(pair, qt, nchunks, pending_muls)
                prev = (pair, qt, ctx2)

            # ---- tail: last unit's normalize in halves + o-rounds ----------
            # (any leftover dep-ready fillers first, so they precede the
            # normalize chain in each engine's in-order queue)
            while filler_q:
                emit_rid(filler_q.pop(0))
            lpair, lqt, lctx2 = prev
            for c0 in (0, 256):
                rbrs = [norm_phase_a(lpair, hh, lqt, lctx2[hh], c0, 256)
                        for hh in range(2)]
                for hh in range(2):
                    norm_phase_b(lpair, hh, lqt, lctx2[hh], rbrs[hh], c0, 256)
                for nb in range(4 * lqt + c0 // 128, 4 * lqt + c0 // 128 + 2):
                    emit_rid(("o", nb, 0))
                    emit_rid(("o", nb, 1))

    nc.compile()
    return nc


_PROGRAM_CACHE = {}


def kernel(x, attention_mask, W_Q, W_K, W_V, W_out, b_out):
    global LAST_RESULTS
    from concourse.bass_utils import run_bass_kernel_spmd

    x = np.ascontiguousarray(x, dtype=np.float32)
    attention_mask = np.asarray(attention_mask)
    lengths = attention_mask.astype(np.int64).sum(axis=1)
    kb_max = int(math.ceil(lengths.max() / KBLK))
    jpad_min = int(lengths.min() // KBLK)

    key = (kb_max, jpad_min)
    if key not in _PROGRAM_CACHE:
        _PROGRAM_CACHE[key] = _build_program(kb_max, jpad_min)
    nc = _PROGRAM_CACHE[key]

    # host-side input prep (matmul operands pre-cast to bf16)
    import ml_dtypes
    BF = ml_dtypes.bfloat16
    xT = [np.ascontiguousarray(x[b].T.astype(BF)) for b in range(B)]
    wqT = np.ascontiguousarray(np.asarray(W_Q, dtype=np.float32).T.astype(BF))
    wkT = np.ascontiguousarray(np.asarray(W_K, dtype=np.float32).T.astype(BF))
    wvT = np.ascontiguousarray(np.asarray(W_V, dtype=np.float32).T.astype(BF))
    woT = np.ascontiguousarray(np.asarray(W_out, dtype=np.float32).T.astype(BF))
    # padbias[p, j] = 0 if key j*128+p is real else NEGB
    padb = [
        np.ascontiguousarray(
            np.where(attention_mask[b].reshape(16, 128).T != 0, 0.0, NEGB)
        ).astype(np.float32)
        for b in range(B)
    ]
    # trineg[p, u] = NEG if u < p + 384 else 0; slice [384-128d : 896-128d]
    # gives the causal additive mask for a diagonal block with offset 128d.
    pp = np.arange(128)[:, None]
    uu = np.arange(896)[None, :]
    trineg = np.where(uu < pp + 384, NEG, 0.0).astype(BF)

    in_maps = []
    for c in range(NCORES):
        b, g = divmod(c, 4)
        sl = slice(g * 256, (g + 1) * 256)
        in_maps.append(
            {
                "xt": xT[b],
                "wq": np.ascontiguousarray(wqT[:, sl]),
                "wk": np.ascontiguousarray(wkT[:, sl]),
                "wv": np.ascontiguousarray(wvT[:, sl]),
                "wout": np.ascontiguousarray(woT[sl, :]),
                "padbias": padb[b],
                "trineg": trineg,
            }
        )

    trace = bool(int(os.environ.get("KERNEL_TRACE", "0")))
    ncores_run = int(os.environ.get("KERNEL_NCORES", str(NCORES)))
    res = run_bass_kernel_spmd(
        nc,
        in_maps[:ncores_run],
        core_ids=list(range(ncores_run)),
        trace=trace,
        trace_cores=list(range(ncores_run)) if trace else None,
    )
    LAST_RESULTS = res

    out = np.zeros((B, N, D), dtype=np.float32)
    for c in range(len(res.results)):
        out[c // 4] += np.asarray(res.results[c]["out"], dtype=np.float32)
    out += np.asarray(b_out, dtype=np.float32)[None, None, :]
    return out


# revision 7
# speedup vs baseline: 1.2509x; 1.2509x over previous
"""Trainium2 Bass kernel for causal+padded multi-head attention.

Problem: B=2, N=2048, D=1024, H=16 heads (DK=64), fp32 I/O.
  out = softmax(mask(x Wq^T (x Wk^T)^T) / sqrt(DK)) (x Wv^T) Wout^T + b_out

Sharding (8 cores): core c handles batch b=c//4 and heads [4*(c%4), 4*(c%4)+4).
Each core computes a partial output [N, D] (its 4 heads' contribution through
the output projection, bf16); the host sums the 4 partials per batch in fp32
and adds b_out.

Schedule (v2, from the 156us baseline's trace):
  - PE p-state: TRN2 throttles the PE clock to 1.2/0.65 GHz after idle and
    only reaches 2.4 GHz after ~3us of continuous execution.  A chain of
    warm-up matmuls on a memset tile runs during the DMA head so real work
    starts at full clock, and the schedule keeps the PE dense to stay there.
  - Input DMAs are split per-128-row chunk (wq/wk/wv per e, xt per (e, qtile))
    and interleaved across BOTH hardware queues (SP + ACT) so the first
    projection matmul can start ~1us after the queues spin up instead of
    after the full weight load.
  - Attention units (pair, qtile) riffle their OWN PV matmuls into the S^T
    stream at a small lag (the PSUM ctx banks of the previous unit are
    released by its normalize, which is emitted just before this unit).
    Unit order ends on a 4-chunk qt=0 unit so the tail chain after the last
    exp is short.
  - Normalize is split into phases: the DVE transpose/reciprocal dance at the
    unit boundary, the GpSimd partition-broadcast right after, and the DVE
    multiplies flushed 2 chunks into the next unit's stream -- so the 1us
    broadcast latency never stalls the in-order DVE queue in front of the
    round evacuations that pace the PE.
  - The LAST unit's normalize runs in 256-column halves with the final
    out-projection rounds interleaved, and the final out DMAs are split per
    512-column half and alternated across both queues.

Known dead ends (measured): fp8 anywhere gives 4-8e-2 rel err (gate 1e-2);
reciprocal_approx_fast (custom DVE uop) returns garbage/crashes the exec unit
under this runtime; exp(-ln(den)) on ScalarE forces an ACT table swap per
call (~38us total); SBUF->SBUF DMA cannot cross partitions, and DRAM-bounce
transposes race (DMA queue issue is async, Tile does not serialize the DRAM
RAW).
"""

import math
import os

import numpy as np

B, N, D, H = 2, 2048, 1024, 16
DK = D // H  # 64
NCORES = 8
HEADS_PER_CORE = 4
QTILE = 512
KBLK = 128
NEG = -30000.0
NEGB = -3750.0  # pad bias applied after the 0.125 scale inside exp
SCALE = 1.0 / math.sqrt(float(DK))  # 0.125
RIFFLE_LAG = 3  # PV chunk j rides the same unit's S^T stream after chunk j+LAG
NWARM = 10  # PE clock warm-up matmuls during the DMA head

# Set by run() when tracing is enabled (test.py reads this).
LAST_RESULTS = None


def _build_program(kb_max: int, jpad_min: int):
    import concourse.tile as tile
    from concourse import bacc, mybir

    F32 = mybir.dt.float32
    BF16 = mybir.dt.bfloat16
    EXP = mybir.ActivationFunctionType.Exp
    ADD = mybir.AluOpType.add

    nc = bacc.Bacc(None)

    xt_d = nc.dram_tensor("xt", [D, N], BF16, kind="ExternalInput")
    wq_d = nc.dram_tensor("wq", [D, 256], BF16, kind="ExternalInput")
    wk_d = nc.dram_tensor("wk", [D, 256], BF16, kind="ExternalInput")
    wv_d = nc.dram_tensor("wv", [D, 256], BF16, kind="ExternalInput")
    wout_d = nc.dram_tensor("wout", [256, D], BF16, kind="ExternalInput")
    padb_d = nc.dram_tensor("padbias", [128, 16], F32, kind="ExternalInput")
    trineg_d = nc.dram_tensor("trineg", [128, 896], BF16, kind="ExternalInput")
    out_d = nc.dram_tensor("out", [N, D], BF16, kind="ExternalOutput")

    NB = N // KBLK  # 16 key/row blocks
    NQT = N // QTILE  # 4 q tiles

    with tile.TileContext(nc) as tc:
        with (
            tc.tile_pool(name="w", bufs=1) as w_pool,
            tc.tile_pool(name="big", bufs=1) as big_pool,
            tc.tile_pool(name="work", bufs=3) as work_pool,
            tc.tile_pool(name="osb", bufs=3) as osb_pool,
            tc.tile_pool(name="xt", bufs=1) as xt_pool,
            tc.tile_pool(name="pt", bufs=10) as pt_pool,
            tc.tile_pool(name="ps_st", bufs=2, space="PSUM") as ps_st,
            tc.tile_pool(name="ps_b", bufs=2, space="PSUM") as ps_b,
            tc.tile_pool(name="ps_ctx", bufs=1, space="PSUM") as ps_ctx,
        ):
            # ---- SBUF tiles ------------------------------------------------
            # weights/x split in e-halves: [128, 4 rows-of-128, cols].  One
            # dma_start costs ~0.7us of ISSUING-ENGINE time (and blocks on
            # ring backlog), so transfers are few and large; halves keep the
            # first matmul's dependency at 256KB instead of 512KB.
            wqh = [w_pool.tile([128, 4, 256], BF16, tag=f"wqh{h}", name=f"wqh{h}") for h in range(2)]
            wkh = [w_pool.tile([128, 4, 256], BF16, tag=f"wkh{h}", name=f"wkh{h}") for h in range(2)]
            wvh = [w_pool.tile([128, 4, 256], BF16, tag=f"wvh{h}", name=f"wvh{h}") for h in range(2)]
            wo2 = [w_pool.tile([128, D], BF16, tag=f"wo{c}", name=f"wo{c}") for c in range(2)]
            padb_t = w_pool.tile([128, 16], F32, tag="padb", name="padb")
            trineg_t = w_pool.tile([128, 896], BF16, tag="trineg", name="trineg")
            xth = [[xt_pool.tile([128, 4, 512], BF16, tag=f"xt{c}_{h}", name=f"xt{c}_{h}")
                    for h in range(2)] for c in range(NQT)]

            def wq8(e):
                return wqh[e // 4][:, e % 4, :]

            def wk8(e):
                return wkh[e // 4][:, e % 4, :]

            def wv8(e):
                return wvh[e // 4][:, e % 4, :]

            def xt(e, c):
                return xth[c][e // 4][:, e % 4, :]

            # PE clock warm-up: a dense chain of matmuls on a memset tile
            # runs during the DMA head so the p-state ramp (0.65 -> 1.2 ->
            # 2.4 GHz, evaluated in ~4us HAM windows) finishes before the
            # first real matmul.
            warmw = w_pool.tile([128, 512], BF16, tag="warmw", name="warmw")
            nc.vector.memset(warmw[:], 0.03)
            for _ in range(NWARM):
                wps = ps_b.tile([128, 512], F32, tag="b", name="b")
                nc.tensor.matmul(wps[:], warmw[:, 0:128], warmw[:],
                                 start=True, stop=True)

            # ---- input DMAs ------------------------------------------------
            # sync queue carries the bulk; the scalar (ACT) queue gets ONLY
            # the three transfers needed before the first exp, then the warm
            # exp -- everything after would delay the exp stream (each
            # dma_start blocks the engine for max(0.7us, ring backlog)).
            def _ld(q, dst, dram, h, cols):
                q.dma_start(
                    dst[:],
                    dram[h * 512:(h + 1) * 512, :].rearrange(
                        "(e p) m -> p e m", p=128
                    ) if cols is None else
                    dram[h * 512:(h + 1) * 512, cols[0]:cols[1]].rearrange(
                        "(e p) m -> p e m", p=128
                    ),
                )

            _ld(nc.sync, wqh[0], wq_d, 0, None)
            _ld(nc.scalar, wqh[1], wq_d, 1, None)
            _ld(nc.sync, xth[0][0], xt_d, 0, (0, 512))
            _ld(nc.scalar, xth[0][1], xt_d, 1, (0, 512))
            _ld(nc.sync, wkh[0], wk_d, 0, None)
            _ld(nc.scalar, wkh[1], wk_d, 1, None)
            nc.sync.dma_start(trineg_t[:], trineg_d[:])

            # warm the ACT exp table now: the scalar engine has issued its 3
            # DMAs; the ~1.5us table load overlaps their transfers
            warm = work_pool.tile([1, 8], F32, tag="warm", name="warm")
            nc.vector.memset(warm[:], 1.0)
            nc.scalar.activation(warm[:], warm[:], EXP)

            _ld(nc.sync, wvh[0], wv_d, 0, None)
            _ld(nc.scalar, wvh[1], wv_d, 1, None)
            for c in range(1, NQT):
                _ld(nc.sync, xth[c][0], xt_d, 0, (c * 512, (c + 1) * 512))
                _ld(nc.scalar if c < 3 else nc.sync,
                    xth[c][1], xt_d, 1, (c * 512, (c + 1) * 512))
            nc.sync.dma_start(wo2[0][:], wout_d[0:128, :])
            nc.sync.dma_start(wo2[1][:], wout_d[128:256, :])
            nc.sync.dma_start(padb_t[:], padb_d[:])

            # V' tile: [keys 128, key-block, head 4, 65]; col 64 <- ones so
            # P@V' also yields the softmax denominator on ctx row 64.
            v4 = big_pool.tile([128, kb_max, 4, 65], BF16, tag="v4", name="v4")
            nc.gpsimd.memset(v4[:, :, :, 64:65], 1.0)

            qt_pair = [big_pool.tile([128, N], BF16, tag=f"qt{p}", name=f"qt{p}") for p in range(2)]
            kt_pair = [big_pool.tile([128, N], BF16, tag=f"kt{p}", name=f"kt{p}") for p in range(2)]
            ctx_pair = [big_pool.tile([128, N], BF16, tag=f"ctx{p}", name=f"ctx{p}") for p in range(2)]

            # ---- PE filler rounds (projections / V / out-projection) -------
            pe_ns = [0.0]  # emitted PE work (ns)
            act_ns = [0.0]  # emitted ACT work (ns)

            def qk_round(w8, pair, nq, dst):
                ps = ps_b.tile([128, 512], F32, tag="b", name="b")
                for e in range(8):
                    nc.tensor.matmul(
                        ps[:],
                        w8(e)[:, pair * 128:(pair + 1) * 128],
                        xt(e, nq),
                        start=(e == 0),
                        stop=(e == 7),
                    )
                nc.vector.tensor_copy(dst[pair][:, nq * 512:(nq + 1) * 512], ps[:])
                pe_ns[0] += 8 * 512 / 2.4

            def v_round(nb):
                ps = ps_b.tile([128, 512], F32, tag="b", name="b")[:, 0:256]
                c, coff = divmod(nb, 4)
                for e in range(8):
                    nc.tensor.matmul(
                        ps[:],
                        xt(e, c)[:, coff * 128:(coff + 1) * 128],
                        wv8(e),
                        start=(e == 0),
                        stop=(e == 7),
                    )
                nc.vector.tensor_copy(
                    v4[:, nb, :, 0:64], ps[:].rearrange("p (h d) -> p h d", h=4)
                )
                pe_ns[0] += 8 * 256 / 2.4

            osb_tiles = {}

            def o_round(nb, fc):
                if fc == 0:
                    osb_tiles[nb] = osb_pool.tile([128, D], BF16, tag="osb", name="osb")
                osb = osb_tiles[nb]
                ps = ps_b.tile([128, 512], F32, tag="b", name="b")
                for pr2 in range(2):
                    nc.tensor.matmul(
                        ps[:],
                        ctx_pair[pr2][:, nb * 128:(nb + 1) * 128],
                        wo2[pr2][:, fc * 512:(fc + 1) * 512],
                        start=(pr2 == 0),
                        stop=(pr2 == 1),
                    )
                nc.vector.tensor_copy(osb[:, fc * 512:(fc + 1) * 512], ps[:])
                # per-half DMA on alternating queues: smaller final drain
                q = nc.sync if (nb + fc) % 2 == 0 else nc.scalar
                q.dma_start(
                    out_d[nb * 128:(nb + 1) * 128, fc * 512:(fc + 1) * 512],
                    osb[:, fc * 512:(fc + 1) * 512],
                )
                if fc == 1:
                    del osb_tiles[nb]
                pe_ns[0] += 2 * 512 / 2.4

            rounds = {}
            for pair in range(2):
                for nq in range(NQT):
                    rounds[("q", pair, nq)] = (lambda p=pair, n=nq: qk_round(wq8, p, n, qt_pair))
                    rounds[("k", pair, nq)] = (lambda p=pair, n=nq: qk_round(wk8, p, n, kt_pair))
            for nb in range(kb_max):
                rounds[("v", nb)] = (lambda n=nb: v_round(n))
            for nb in range(NB):
                for fc in range(2):
                    rounds[("o", nb, fc)] = (lambda n=nb, f=fc: o_round(n, f))

            emitted = set()
            filler_q = []

            def emit_rid(rid):
                if rid in emitted:
                    return
                emitted.add(rid)
                rounds[rid]()

            def inject_fillers(headroom=4000.0):
                # keep ~4us of emitted-but-unexecuted PE work beyond the ACT
                # frontier so the PE (the critical engine) never drains
                while filler_q and pe_ns[0] < act_ns[0] + headroom:
                    emit_rid(filler_q.pop(0))

            # ---- normalize -------------------------------------------------
            # The DVE reciprocal is an iterative 8-cyc/element divide
            # streaming the FREE dim, and the denominator row is 512 elements
            # on ONE partition.  Use the DVE 32x32 StreamTranspose to fold
            # the row onto 32 partitions, take the reciprocal 16-wide, fold
            # back (bf16: ~0.4% rms on the normalize scale), partition-
            # broadcast on GpSimd, and multiply ctx (read straight from
            # PSUM) by the broadcast row.
            def norm_phase_a(pair, hh, qt, ctx_ps, c0, cw):
                """transpose/recip/transpose + gpsimd broadcast for columns
                [c0, c0+cw); returns the rbr tile for the multiply."""
                nblk = cw // 32
                tscat = work_pool.tile([32, 512], F32, tag="tscat", name="tscat")
                nc.vector.transpose(tscat[:, 0:cw], ctx_ps[64:96, c0:c0 + cw])
                rscat = work_pool.tile([32, 512], BF16, tag="rscat", name="rscat")
                with nc.allow_low_precision(
                    reason="bf16 softmax-denominator reciprocal: ~0.4% rms "
                    "on the normalize scale, inside the error budget"
                ):
                    nc.vector.reciprocal(
                        rscat[:, 0:cw].rearrange("p (b s) -> p b s", s=32)[:, :, 0],
                        tscat[:, 0:cw].rearrange("p (b s) -> p b s", s=32)[:, :, 0],
                    )
                rrow = work_pool.tile([32, 512], BF16, tag="rrow", name="rrow")
                nc.vector.transpose(rrow[:, 0:cw], rscat[:, 0:cw])
                act_ns[0] += 2 * (cw + 352) / 1.2
                rbr = work_pool.tile([64, 512], BF16, tag="rbr", name="rbr")
                # GpSimd runs ONLY partition_broadcast ops (+ the startup
                # memsets): op-type churn makes walrus swap the firmware
                # library (~7us per swap)
                nc.gpsimd.partition_broadcast(rbr[:, 0:cw], rrow[0:1, 0:cw])
                return rbr

            def norm_phase_b(pair, hh, qt, ctx_ps, rbr, c0, cw):
                hp = slice(64 * hh, 64 * hh + 64)
                nc.vector.tensor_mul(
                    ctx_pair[pair][hp, qt * 512 + c0:qt * 512 + c0 + cw],
                    ctx_ps[0:64, c0:c0 + cw],
                    rbr[:, 0:cw],
                )

            done_norms = {q: 0 for q in range(NQT)}

            def note_norm_done(nqt):
                done_norms[nqt] += 1
                if done_norms[nqt] == 2:
                    for nb in range(4 * nqt, 4 * nqt + 4):
                        filler_q.append(("o", nb, 0))
                        filler_q.append(("o", nb, 1))

            # ---- attention unit: S^T + exp stream with own PV riffled ------
            def emit_unit(pair, qt, nchunks, pending_muls):
                ctx2 = [
                    ps_ctx.tile([96, 512], F32, tag=f"ctx{hh}", name=f"ctx{hh}")
                    for hh in range(2)
                ]
                pvq = []

                def pv_chunk(j, ptt, off):
                    for hh in range(2):
                        nc.tensor.matmul(
                            ctx2[hh][0:65, off:],
                            v4[:, j, 2 * pair + hh, :],
                            ptt[:, hh, off:],
                            start=(j == 0),
                            stop=(j == nchunks - 1),
                            skip_group_check=True,
                        )
                    pe_ns[0] += 2 * (512 - off) / 2.4

                for j in range(nchunks):
                    if j == 2 and pending_muls:
                        # prev unit's normalize multiplies, flushed after the
                        # gpsimd broadcasts have had ~2 chunks to complete so
                        # the in-order DVE queue never stalls on them
                        for fn in pending_muls:
                            fn()
                        pending_muls.clear()
                    while pvq and pvq[0][0] <= j - RIFFLE_LAG:
                        pv_chunk(*pvq.pop(0))
                    inject_fillers()
                    d = j - 4 * qt
                    # exact-causal column trim (keep matmul N >= 128)
                    off = 128 * d if d >= 1 else 0
                    st_ps = ps_st.tile([128, 2, 512], F32, tag="blk", name="blk")
                    for hh in range(2):
                        hp = slice(64 * hh, 64 * hh + 64)
                        nc.tensor.matmul(
                            st_ps[:, hh, off:],
                            kt_pair[pair][hp, j * 128:(j + 1) * 128],
                            qt_pair[pair][hp, qt * 512 + off:(qt + 1) * 512],
                            start=True,
                            stop=True,
                        )
                    pe_ns[0] += (512 - off) / 2.4
                    if d >= 0:
                        # causal add -30000; with off = 128*d the masked
                        # triangle lies entirely in cols [off, off+128);
                        # one op covers both heads via a stride-0 broadcast
                        u0 = 384 - 128 * d + off
                        w = min(128, 512 - off)
                        nc.vector.tensor_tensor(
                            st_ps[:, :, off:off + w],
                            st_ps[:, :, off:off + w],
                            trineg_t[:, u0:u0 + w].unsqueeze(1).broadcast_to(
                                (128, 2, w)
                            ),
                            ADD,
                        )
                    pt_t = pt_pool.tile([128, 2, 512], BF16, tag="pt", name="pt")
                    kw = {}
                    if j >= jpad_min:  # per-key pad bias (same for both heads)
                        kw["bias"] = padb_t[:, j:j + 1]
                    nc.scalar.activation(
                        pt_t[:, :, off:], st_ps[:, :, off:], EXP, scale=SCALE, **kw
                    )
                    act_ns[0] += (2 * (512 - off) + 352) / 1.2
                    pvq.append((j, pt_t, off))
                if pending_muls:
                    for fn in pending_muls:
                        fn()
                    pending_muls.clear()
                while pvq:
                    pv_chunk(*pvq.pop(0))
                return ctx2

            # Unit order: start with the cheapest unit (least input DMA),
            # end with a 4-chunk unit so the post-last-exp tail (PV drain +
            # normalize + o-rounds + out DMA) is short.
            units = [(0, 0), (1, 0), (0, 1), (0, 3), (1, 3), (0, 2), (1, 2), (1, 1)]
            units = [(p, qt, min(4 * qt + 4, kb_max)) for (p, qt) in units]

            # projection/V rounds become filler, ordered by the deadline of
            # the unit that first needs them (emit_rid dedups, so rounds the
            # unit loop hard-emits are simply skipped here)
            seen_rounds = set()
            for (p, qt, nch) in units:
                for rid in ([("q", p, qt)]
                            + [("k", p, nq) for nq in range(qt + 1)]
                            + [("v", nb) for nb in range(nch)]):
                    if rid not in seen_rounds:
                        seen_rounds.add(rid)
                        filler_q.append(rid)

            pending_muls = []
            prev = None  # (pair, qt, ctx2)
            for idx, (pair, qt, nchunks) in enumerate(units):
                # HARD-emit this unit's projection/V prereqs before any of
                # its S^T/PV instructions.  A read emitted before its writer
                # gets NO dependency from the Tile tracker (emission-ordered)
                # and would consume uninitialized SBUF; filler pacing alone
                # must never be trusted for correctness.
                emit_rid(("q", pair, qt))
                for nq in range(qt + 1):
                    emit_rid(("k", pair, nq))
                for nb in range(nchunks):
                    emit_rid(("v", nb))
                if prev is not None:
                    # previous unit's normalize: dance+broadcast now, the
                    # multiplies 2 chunks into this unit's stream.  This MUST
                    # precede emit_unit: ps_ctx has bufs=1, so this unit's PV
                    # start-write reuses the previous ctx banks and the Tile
                    # tracker (emission-ordered) only serializes them if the
                    # normalize reads are emitted first.
                    ppair, pqt, pctx2 = prev
                    for hh in range(2):
                        rbr = norm_phase_a(ppair, hh, pqt, pctx2[hh], 0, 512)
                        pending_muls.append(
                            (lambda p=ppair, h=hh, q=pqt, c=pctx2[hh], r=rbr:
                             norm_phase_b(p, h, q, c, r, 0, 512))
                        )
                    pending_muls.append(lambda q=pqt: note_norm_done(q))
                ctx2 = emit_unit(pair, qt, nchunks, pending_muls)
                prev = (pair, qt, ctx2)

            # ---- tail: last unit's normalize in halves + o-rounds ----------
            # (any leftover dep-ready fillers first, so they precede the
            # normalize chain in each engine's in-order queue)
            while filler_q:
                emit_rid(filler_q.pop(0))
            lpair, lqt, lctx2 = prev
            for c0 in (0, 256):
                rbrs = [norm_phase_a(lpair, hh, lqt, lctx2[hh], c0, 256)
                        for hh in range(2)]
                for hh in range(2):
                    norm_phase_b(lpair, hh, lqt, lctx2[hh], rbrs[hh], c0, 256)
                for nb in range(4 * lqt + c0 // 128, 4 * lqt + c0 // 128 + 2):
                    emit_rid(("o", nb, 0))
                    emit_rid(("o", nb, 1))

    nc.compile()
    return nc


_PROGRAM_CACHE = {}


def kernel(x, attention_mask, W_Q, W_K, W_V, W_out, b_out):
    global LAST_RESULTS
    from concourse.bass_utils import run_bass_kernel_spmd

    x = np.ascontiguousarray(x, dtype=np.float32)
    attention_mask = np.asarray(attention_mask)
    lengths = attention_mask.astype(np.int64).sum(axis=1)
    kb_max = int(math.ceil(lengths.max() / KBLK))
    jpad_min = int(lengths.min() // KBLK)

    key = (kb_max, jpad_min)
    if key not in _PROGRAM_CACHE:
        _PROGRAM_CACHE[key] = _build_program(kb_max, jpad_min)
    nc = _PROGRAM_CACHE[key]

    # host-side input prep (matmul operands pre-cast to bf16)
    import ml_dtypes
    BF = ml_dtypes.bfloat16
    xT = [np.ascontiguousarray(x[b].T.astype(BF)) for b in range(B)]
    wqT = np.ascontiguousarray(np.asarray(W_Q, dtype=np.float32).T.astype(BF))
    wkT = np.ascontiguousarray(np.asarray(W_K, dtype=np.float32).T.astype(BF))
    wvT = np.ascontiguousarray(np.asarray(W_V, dtype=np.float32).T.astype(BF))
    woT = np.ascontiguousarray(np.asarray(W_out, dtype=np.float32).T.astype(BF))
    # padbias[p, j] = 0 if key j*128+p is real else NEGB
    padb = [
        np.ascontiguousarray(
            np.where(attention_mask[b].reshape(16, 128).T != 0, 0.0, NEGB)
        ).astype(np.float32)
        for b in range(B)
    ]
    # trineg[p, u] = NEG if u < p + 384 else 0; slice [384-128d : 896-128d]
    # gives the causal additive mask for a diagonal block with offset 128d.
    pp = np.arange(128)[:, None]
    uu = np.arange(896)[None, :]
    trineg = np.where(uu < pp + 384, NEG, 0.0).astype(BF)

    in_maps = []
    for c in range(NCORES):
        b, g = divmod(c, 4)
        sl = slice(g * 256, (g + 1) * 256)
        in_maps.append(
            {
                "xt": xT[b],
                "wq": np.ascontiguousarray(wqT[:, sl]),
                "wk": np.ascontiguousarray(wkT[:, sl]),
                "wv": np.ascontiguousarray(wvT[:, sl]),
                "wout": np.ascontiguousarray(woT[sl, :]),
                "padbias": padb[b],
                "trineg": trineg,
            }
        )

    trace = bool(int(os.environ.get("KERNEL_TRACE", "0")))
    ncores_run = int(os.environ.get("KERNEL_NCORES", str(NCORES)))
    res = run_bass_kernel_spmd(
        nc,
        in_maps[:ncores_run],
        core_ids=list(range(ncores_run)),
        trace=trace,
        trace_cores=list(range(ncores_run)) if trace else None,
    )
    LAST_RESULTS = res

    out = np.zeros((B, N, D), dtype=np.float32)
    for c in range(len(res.results)):
        out[c // 4] += np.asarray(res.results[c]["out"], dtype=np.float32)
    out += np.asarray(b_out, dtype=np.float32)[None, None, :]
    return out
